# revision 2
# baseline (speedup 1.0000x reference)
"""Single-head attention (B=4, S=2048, D=1024) on 8 TRN2 NeuronCores.

Sharding: 8 shards = (batch b, query-half h).  Core c = 2*b + h computes
attention outputs for query rows [h*1024, (h+1)*1024) of batch b.  Each core
computes full K/V for its batch (duplicated across the pair of cores sharing
a batch) plus Q for its own half.  The host permutes x per core so the core's
query rows are always rows [0, 1024) — keys are a permutation of the batch's
rows, which softmax-attention is invariant to — so a single SPMD NEFF serves
all 8 cores.

Per-core dataflow (all matmuls bf16 in / fp32 PSUM accumulate):
  xT  = transpose(cast_bf16(x))                     (DMA-transpose via xbar)
  QT[e,q] = Wq[d,e].T-contract  xT[d,q]             (PE)
  KT[e,k], V[k,e] likewise
  S^T[k,q] = KT[e,k].T-contract QT[e,q]             (PE)
  expS = exp(S^T / sqrt(D))                         (ACT, no max-subtraction:
                                                     |scores| < ~2.5 by input scaling)
  O[q,e]  = expS[k,q].T-contract V[k,e]             (PE)
  denom[q] = expS[k,q].T-contract ones[k,1]         (PE, fused in same loop)
  out = O * (1/denom)                               (DVE)
"""

import numpy as np

_P = 128


def _build_attention_nc(SQ, S, D, n_cores):
    """Build + compile the per-core Bass module.

    SQ: query rows per core, S: key rows (= batch seq len), D: model dim.
    """
    from contextlib import ExitStack

    import concourse.tile as tile
    import concourse.mybir as mybir
    from concourse import bacc

    f32 = mybir.dt.float32
    bf16 = mybir.dt.bfloat16

    DT = D // _P    # contraction tiles over d or e
    ST = S // _P    # key tiles
    QCW = min(512, SQ)   # q chunk width (matmul moving dim)
    KCW = min(512, S)    # k chunk width
    ECW = min(512, D)    # e chunk width
    NQ = SQ // QCW
    NK = S // KCW
    NE = D // ECW
    inv_sqrt_d = 1.0 / float(np.sqrt(D))

    nc = bacc.Bacc(
        "TRN2",
        target_bir_lowering=False,
        debug=False,
        enable_asserts=True,
        num_devices=n_cores,
    )
    x_ap = nc.dram_tensor("x", [S, D], f32, kind="ExternalInput").ap()
    w_ap = nc.dram_tensor("w", [3, D, D], f32, kind="ExternalInput").ap()
    out_ap = nc.dram_tensor("out", [SQ, D], f32, kind="ExternalOutput").ap()

    with ExitStack() as ctx:
        tc = ctx.enter_context(tile.TileContext(nc))

        pers = ctx.enter_context(tc.tile_pool(name="pers", bufs=1))
        QT = pers.tile([_P, DT, SQ], bf16)      # Q^T: [e_inner, e_tile, q]
        KT = pers.tile([_P, DT, S], bf16)       # K^T: [e_inner, e_tile, k]
        V = pers.tile([_P, ST, D], bf16)        # V:   [k_inner, k_tile, e]
        ones = pers.tile([_P, 1], bf16)
        nc.vector.memset(ones, 1.0)

        outp = ctx.enter_context(tc.tile_pool(name="outp", bufs=2))
        psum = ctx.enter_context(tc.tile_pool(name="psum", bufs=5, space="PSUM"))
        psum_dn = ctx.enter_context(tc.tile_pool(name="psum_dn", bufs=2, space="PSUM"))

        with tc.tile_pool(name="big", bufs=1) as big, tc.tile_pool(
            name="ld", bufs=2
        ) as ld:
            # x^T: [d_inner, s_tile, d_tile, s_inner]; s run for chunk c of
            # width QCW/KCW is xT[:, c*w//128:(c+1)*w//128, dt, :].
            xT = big.tile([_P, ST, DT, _P], bf16)
            Wb = big.tile([_P, 3, DT, D], bf16)  # [d_inner, qkv, d_tile, e]

            for i in range(3):
                for dt in range(DT):
                    w_nat = ld.tile([_P, D], f32, tag="w_nat")
                    nc.sync.dma_start(out=w_nat, in_=w_ap[i, dt * _P : (dt + 1) * _P, :])
                    nc.vector.tensor_copy(out=Wb[:, i, dt, :], in_=w_nat)

            for st in range(ST):
                x_nat = ld.tile([_P, D], f32, tag="x_nat")
                nc.sync.dma_start(out=x_nat, in_=x_ap[st * _P : (st + 1) * _P, :])
                x_bf = ld.tile([_P, D], bf16, tag="x_bf")
                nc.vector.tensor_copy(out=x_bf, in_=x_nat)
                # out[p, m, n] = in[n, m*128 + p]  -> xT[p, st, dt, si] = x[st*128+si, dt*128+p]
                nc.sync.dma_start_transpose(out=xT[:, st, :, :], in_=x_bf)

            # ---- Q^T and K^T projections: out[e, s] = W[d, e].T @ xT[d, s]
            for which, dst, ncw, cw in ((0, QT, NQ, QCW), (1, KT, NK, KCW)):
                for et in range(DT):
                    for c in range(ncw):
                        ps = psum.tile([_P, cw], f32, tag="mm", name="ps")
                        for dt in range(DT):
                            nc.tensor.matmul(
                                ps,
                                lhsT=Wb[:, which, dt, et * _P : (et + 1) * _P],
                                rhs=xT[:, c * (cw // _P) : (c + 1) * (cw // _P), dt, :],
                                start=(dt == 0),
                                stop=(dt == DT - 1),
                            )
                        nc.scalar.copy(out=dst[:, et, c * cw : (c + 1) * cw], in_=ps)

            # ---- V: out[s, e] = xT[d, s].T @ W[d, e]
            for st in range(ST):
                for ec in range(NE):
                    ps = psum.tile([_P, ECW], f32, tag="mm", name="ps")
                    for dt in range(DT):
                        nc.tensor.matmul(
                            ps,
                            lhsT=xT[:, st, dt, :],
                            rhs=Wb[:, 2, dt, ec * ECW : (ec + 1) * ECW],
                            start=(dt == 0),
                            stop=(dt == DT - 1),
                        )
                    nc.scalar.copy(out=V[:, st, ec * ECW : (ec + 1) * ECW], in_=ps)

        # big + ld released; expS reuses their space
        with tc.tile_pool(name="es", bufs=2) as es_pool:
            for qc in range(NQ):
                expS = es_pool.tile([_P, ST, QCW], bf16, tag="expS", name="expS")
                # S^T[k, q] = KT[e, k].T @ QT[e, q], accumulated over e tiles
                for kt in range(ST):
                    ps = psum.tile([_P, QCW], f32, tag="mm", name="ps")
                    for et in range(DT):
                        nc.tensor.matmul(
                            ps,
                            lhsT=KT[:, et, kt * _P : (kt + 1) * _P],
                            rhs=QT[:, et, qc * QCW : (qc + 1) * QCW],
                            start=(et == 0),
                            stop=(et == DT - 1),
                        )
                    nc.scalar.activation(
                        out=expS[:, kt, :],
                        in_=ps,
                        func=mybir.ActivationFunctionType.Exp,
                        scale=inv_sqrt_d,
                    )

                # O[q, e] = expS[k, q].T @ V[k, e]; denom fused via ones column
                for qs in range(QCW // _P):
                    o_ps = [
                        psum.tile([_P, ECW], f32, tag="mm", name="o_ps")
                        for _ in range(NE)
                    ]
                    d_ps = psum_dn.tile([_P, 1], f32, tag="dn", name="d_ps")
                    for kt in range(ST):
                        lhsT = expS[:, kt, qs * _P : (qs + 1) * _P]
                        for ec in range(NE):
                            nc.tensor.matmul(
                                o_ps[ec],
                                lhsT=lhsT,
                                rhs=V[:, kt, ec * ECW : (ec + 1) * ECW],
                                start=(kt == 0),
                                stop=(kt == ST - 1),
                            )
                        nc.tensor.matmul(
                            d_ps,
                            lhsT=lhsT,
                            rhs=ones,
                            start=(kt == 0),
                            stop=(kt == ST - 1),
                        )
                    recip = outp.tile([_P, 1], f32, tag="recip", name="recip")
                    nc.vector.reciprocal(out=recip, in_=d_ps)
                    o_sb = outp.tile([_P, D], f32, tag="o_sb", name="o_sb")
                    for ec in range(NE):
                        nc.vector.tensor_scalar_mul(
                            out=o_sb[:, ec * ECW : (ec + 1) * ECW],
                            in0=o_ps[ec],
                            scalar1=recip,
                        )
                    row = qc * QCW + qs * _P
                    nc.sync.dma_start(out=out_ap[row : row + _P, :], in_=o_sb)

    nc.compile()
    return nc


_NC_CACHE = {}


def _get_nc(SQ, S, D, n_cores):
    key = (SQ, S, D, n_cores)
    if key not in _NC_CACHE:
        _NC_CACHE[key] = _build_attention_nc(SQ, S, D, n_cores)
    return _NC_CACHE[key]


def _run(x, w, **run_kwargs):
    """Shard inputs, run the SPMD kernel, gather the full output.

    Returns (out, BassKernelResults)."""
    from concourse import bass_utils

    x = np.ascontiguousarray(np.asarray(x, dtype=np.float32))
    w = np.ascontiguousarray(np.asarray(w, dtype=np.float32))
    B, S, D = x.shape
    n_cores = 8
    halves = n_cores // B
    SQ = S // halves

    nc = _get_nc(SQ, S, D, n_cores)

    in_maps = []
    for c in range(n_cores):
        b, h = divmod(c, halves)
        xb = x[b]
        if h:
            # rotate so this core's query rows come first; key order is a
            # permutation, which attention is invariant to
            xb = np.concatenate([xb[h * SQ :], xb[: h * SQ]], axis=0)
        in_maps.append({"x": np.ascontiguousarray(xb), "w": w})

    res = bass_utils.run_bass_kernel_spmd(
        nc, in_maps, core_ids=list(range(n_cores)), **run_kwargs
    )

    out = np.empty((B, S, D), dtype=np.float32)
    for c in range(n_cores):
        b, h = divmod(c, halves)
        out[b, h * SQ : (h + 1) * SQ] = res.results[c]["out"]
    return out, res


def kernel(x, kernel):
    """Full-input entry point: x (4, 2048, 1024) f32, kernel (3, 1024, 1024) f32.

    Returns (4, 2048, 1024) f32 attention output.
    """
    out, _ = _run(x, kernel)
    return out
